# revision 3
# baseline (speedup 1.0000x reference)
"""Trainium2 Bass kernel for nn_ModelInverse.

Inverts a monotone scalar MLP F (PositiveLinear+Sigmoid stack, arch
[1,64,64,1], +1e-3*x monotonic term) at 2M targets z via the equivalent
of 20 bisection steps.

Approach: the map g(z) = F^{-1}(z) is a smooth, nearly-linear scalar
function fixed by the (runtime) weights.  On device we:
  1. invert F at 64 Chebyshev nodes with a Picard fixed-point iteration
     (F' deviates from 1 by <~10%, so convergence is ~10x per step),
  2. least-squares-fit a degree-10 polynomial in u = 2z-1 through the
     node values (fit operator is a constant pseudo-inverse matrix),
  3. evaluate the polynomial at all 2M z with fused DVE ops.
The result matches the fp32 reference bisection to its own fp32 noise
floor (~3.4e-5 absolute, outputs in (0,1)).

Sharding: pure data parallel over the N axis across 8 cores; the tiny
MLP params and fit constants are replicated; no cross-core communication.
"""

import os
import sys

import numpy as np

for _p in ("/opt/trn_rl_repo", "/root/.axon_site/_ro/trn_rl_repo"):
    if os.path.isdir(_p) and _p not in sys.path:
        sys.path.insert(0, _p)

import concourse.bacc as bacc
import concourse.bass as bass
import concourse.mybir as mybir
import concourse.tile as tile
from concourse.bass_utils import run_bass_kernel_spmd

F32 = mybir.dt.float32
AF = mybir.ActivationFunctionType
OP = mybir.AluOpType

N = 2_000_000
NCORES = 8
P = 128           # SBUF partitions
FREE = 1954       # elements per partition per core; 8*128*1954 = 2,000,896
SHARD = P * FREE  # 250,112 elements per core
NCHUNK = 2        # element-phase chunks (DMA/compute overlap)
FC = FREE // NCHUNK

DEG = 10          # polynomial degree
Q = 64            # Chebyshev nodes
NITER = 8         # Picard iterations
MONO = 1e-3
H = 64


def _host_constants():
    qi = np.arange(Q)
    nodes64 = (np.cos((2 * qi + 1) * np.pi / (2 * Q)) + 1.0) / 2.0  # in (0,1)
    nodes = np.concatenate([nodes64, [0.0, 1.0]]).astype(np.float32)[None, :]  # [1,Q+2]
    V = np.vander(2.0 * nodes64 - 1.0, DEG + 1, increasing=True)    # [Q, DEG+1]
    pinvt = np.ascontiguousarray(np.linalg.pinv(V).T).astype(np.float32)  # [Q, DEG+1]
    eye = np.eye(DEG + 1, dtype=np.float32)
    return nodes, pinvt, eye


def _build_program():
    nc = bacc.Bacc("TRN2", target_bir_lowering=False, debug=False,
                   num_devices=NCORES)

    z_in = nc.dram_tensor("z_in", [P, FREE], F32, kind="ExternalInput")
    out = nc.dram_tensor("out", [P, FREE], F32, kind="ExternalOutput")
    w1t = nc.dram_tensor("w1t", [1, H], F32, kind="ExternalInput")    # pre_w1^T
    w2t = nc.dram_tensor("w2t", [H, H], F32, kind="ExternalInput")    # pre_w2^T
    w3t = nc.dram_tensor("w3t", [H, 1], F32, kind="ExternalInput")    # pre_w3^T
    b1d = nc.dram_tensor("b1d", [H, 1], F32, kind="ExternalInput")
    b2d = nc.dram_tensor("b2d", [H, 1], F32, kind="ExternalInput")
    b3d = nc.dram_tensor("b3d", [1, 1], F32, kind="ExternalInput")
    nodes_d = nc.dram_tensor("nodes", [1, Q + 2], F32, kind="ExternalInput")
    pinvt_d = nc.dram_tensor("pinvt", [Q, DEG + 1], F32, kind="ExternalInput")
    eye_d = nc.dram_tensor("eye", [DEG + 1, DEG + 1], F32, kind="ExternalInput")

    from contextlib import ExitStack
    with tile.TileContext(nc) as tc, ExitStack() as ctx:
        const = ctx.enter_context(tc.tile_pool(name="const", bufs=1))
        work = ctx.enter_context(tc.tile_pool(name="work", bufs=2))
        big = ctx.enter_context(tc.tile_pool(name="big", bufs=2))
        psum = ctx.enter_context(tc.tile_pool(name="psum", bufs=2, space="PSUM"))

        # ---- load params / constants, exponentiate positive weights ----
        w1s = const.tile([1, H], F32)
        nc.sync.dma_start(w1s[:], w1t.ap())
        nc.scalar.activation(w1s[:], w1s[:], AF.Exp)
        w2s = const.tile([H, H], F32)
        nc.sync.dma_start(w2s[:], w2t.ap())
        nc.scalar.activation(w2s[:], w2s[:], AF.Exp)
        w3s = const.tile([H, 1], F32)
        nc.sync.dma_start(w3s[:], w3t.ap())
        nc.scalar.activation(w3s[:], w3s[:], AF.Exp)
        b1s = const.tile([H, 1], F32)
        nc.sync.dma_start(b1s[:], b1d.ap())
        b2s = const.tile([H, 1], F32)
        nc.sync.dma_start(b2s[:], b2d.ap())
        b3s = const.tile([1, 1], F32)
        nc.sync.dma_start(b3s[:], b3d.ap())
        zn = const.tile([1, Q + 2], F32)
        nc.sync.dma_start(zn[:], nodes_d.ap())
        pit = const.tile([Q, DEG + 1], F32)
        nc.sync.dma_start(pit[:], pinvt_d.ap())
        eye = const.tile([DEG + 1, DEG + 1], F32)
        nc.sync.dma_start(eye[:], eye_d.ap())
        ones1 = const.tile([1, 1], F32)
        nc.vector.memset(ones1[:], 1.0)
        onesp = const.tile([1, P], F32)
        nc.vector.memset(onesp[:], 1.0)

        # ---- Picard inversion at the Q nodes (plus endpoints 0,1) ----
        W = Q + 2
        x = work.tile([1, W], F32, tag="x")
        nc.vector.tensor_copy(x[:], zn[:])
        for _ in range(NITER):
            p1 = psum.tile([H, W], F32, tag="ps")
            nc.tensor.matmul(p1[:], lhsT=w1s[:], rhs=x[:])
            h1 = work.tile([H, W], F32, tag="h1")
            nc.scalar.activation(h1[:], p1[:], AF.Sigmoid, bias=b1s[:])
            p2 = psum.tile([H, W], F32, tag="ps")
            nc.tensor.matmul(p2[:], lhsT=w2s[:], rhs=h1[:])
            h2 = work.tile([H, W], F32, tag="h2")
            nc.scalar.activation(h2[:], p2[:], AF.Sigmoid, bias=b2s[:])
            p3 = psum.tile([1, W], F32, tag="ps")
            nc.tensor.matmul(p3[:], lhsT=w3s[:], rhs=h2[:])
            ys = work.tile([1, W], F32, tag="ys")
            nc.scalar.activation(ys[:], p3[:], AF.Sigmoid, bias=b3s[:])
            ax = work.tile([1, W], F32, tag="ax")
            nc.vector.scalar_tensor_tensor(ax[:], x[:], MONO, ys[:],
                                           op0=OP.mult, op1=OP.add)
            rr = work.tile([1, 1], F32, tag="rr")
            nc.vector.tensor_sub(rr[:], ax[0:1, W - 1:W], ax[0:1, W - 2:W - 1])
            ir = work.tile([1, 1], F32, tag="ir")
            nc.vector.reciprocal(ir[:], rr[:])
            fz = work.tile([1, W], F32, tag="fz")
            nc.vector.tensor_scalar(fz[:], ax[:], ax[0:1, W - 2:W - 1], ir[:],
                                    op0=OP.subtract, op1=OP.mult)
            dd = work.tile([1, W], F32, tag="dd")
            nc.vector.tensor_sub(dd[:], fz[:], zn[:])
            xn = work.tile([1, W], F32, tag="xn")
            nc.vector.tensor_sub(xn[:], x[:], dd[:])
            x = work.tile([1, W], F32, tag="x")
            nc.vector.tensor_scalar(x[:], xn[:], 0.0, 1.0,
                                    op0=OP.max, op1=OP.min)

        # ---- polynomial fit: c = PINV @ g, broadcast to all partitions ----
        pg = psum.tile([Q, 1], F32, tag="ps")
        nc.tensor.matmul(pg[:], lhsT=x[0:1, 0:Q], rhs=ones1[:])
        gt = work.tile([Q, 1], F32, tag="gt")
        nc.scalar.copy(gt[:], pg[:])
        pc = psum.tile([DEG + 1, 1], F32, tag="ps")
        nc.tensor.matmul(pc[:], lhsT=pit[:], rhs=gt[:])
        cc = work.tile([DEG + 1, 1], F32, tag="cc")
        nc.scalar.copy(cc[:], pc[:])
        pr = psum.tile([1, DEG + 1], F32, tag="ps")
        nc.tensor.matmul(pr[:], lhsT=cc[:], rhs=eye[:])
        cr = work.tile([1, DEG + 1], F32, tag="cr")
        nc.scalar.copy(cr[:], pr[:])
        pb = psum.tile([P, DEG + 1], F32, tag="ps")
        nc.tensor.matmul(pb[:], lhsT=onesp[:], rhs=cr[:])
        ca = const.tile([P, DEG + 1], F32)
        nc.scalar.copy(ca[:], pb[:])

        # ---- evaluate polynomial at all elements ----
        for i in range(NCHUNK):
            sl = slice(i * FC, (i + 1) * FC)
            zt = big.tile([P, FC], F32, tag="zt")
            nc.sync.dma_start(zt[:], z_in.ap()[:, sl])
            u = big.tile([P, FC], F32, tag="u")
            nc.vector.tensor_scalar(u[:], zt[:], 2.0, -1.0,
                                    op0=OP.mult, op1=OP.add)
            y = big.tile([P, FC], F32, tag="y")
            nc.vector.tensor_scalar(y[:], u[:], ca[:, DEG:DEG + 1], None,
                                    op0=OP.mult)
            for d in range(DEG - 1, 0, -1):
                y2 = big.tile([P, FC], F32, tag="y2")
                nc.vector.scalar_tensor_tensor(y2[:], y[:], ca[:, d:d + 1], u[:],
                                               op0=OP.add, op1=OP.mult)
                y = y2
            yf = big.tile([P, FC], F32, tag="yf")
            nc.vector.tensor_scalar(yf[:], y[:], ca[:, 0:1], None, op0=OP.add)
            nc.sync.dma_start(out.ap()[:, sl], yf[:])

    nc.compile()
    return nc


_NC_CACHE = None


def _get_program():
    global _NC_CACHE
    if _NC_CACHE is None:
        _NC_CACHE = _build_program()
    return _NC_CACHE


def _make_in_maps(z, pre_w1, b1, pre_w2, b2, pre_w3, b3):
    z = np.ascontiguousarray(np.asarray(z, dtype=np.float32).reshape(-1))
    assert z.size == N, z.shape
    zp = np.zeros(NCORES * SHARD, dtype=np.float32)
    zp[:N] = z
    shards = zp.reshape(NCORES, P, FREE)

    f32 = np.float32
    nodes, pinvt, eye = _host_constants()
    common = {
        "w1t": np.ascontiguousarray(np.asarray(pre_w1, f32).reshape(1, H)),
        "w2t": np.ascontiguousarray(np.asarray(pre_w2, f32).T),
        "w3t": np.ascontiguousarray(np.asarray(pre_w3, f32).reshape(H, 1)),
        "b1d": np.ascontiguousarray(np.asarray(b1, f32).reshape(H, 1)),
        "b2d": np.ascontiguousarray(np.asarray(b2, f32).reshape(H, 1)),
        "b3d": np.ascontiguousarray(np.asarray(b3, f32).reshape(1, 1)),
        "nodes": nodes, "pinvt": pinvt, "eye": eye,
    }
    return [dict(common, z_in=np.ascontiguousarray(shards[i]))
            for i in range(NCORES)]


def kernel(z, pre_w1, b1, pre_w2, b2, pre_w3, b3):
    in_maps = _make_in_maps(z, pre_w1, b1, pre_w2, b2, pre_w3, b3)
    nc = _get_program()
    res = run_bass_kernel_spmd(nc, in_maps, list(range(NCORES))).results
    out = np.concatenate([np.asarray(res[i]["out"], dtype=np.float32).reshape(-1)
                          for i in range(NCORES)])[:N]
    return out.reshape(N, 1)


def profile_once(inputs):
    """Run once with tracing and return HW exec time in ns (test helper)."""
    in_maps = _make_in_maps(**inputs)
    nc = _get_program()
    r = run_bass_kernel_spmd(nc, in_maps, list(range(NCORES)), trace=True)
    return r.exec_time_ns
